# revision 17
# baseline (speedup 1.0000x reference)
"""Pairwise squared-euclidean-distance kernel (-log1p(max(d2,0))) for 8 trn2 cores.

Strategy (sharding_hint): shard x1 rows across the 8 NeuronCores (1024 rows
each); replicate x2. Each core computes a [1024, 8192] slab of the output:

    out[n, m] = -log1p(sq1[n] + sq2[m] - 2 * x1[n] . x2[m])

Device work per core: a [1024 x 1024] @ [1024 x 8192] matmul into PSUM
(psum = -2 * cross, the -2 baked into the lhsT operand on the host; fp8 e4m3
operands, DoubleRowSwInterleave so each 512-col pass covers 256 contraction
rows -- the HW-max rate of 1 moving column/cycle at 2.4 GHz, ~216 ns/pass),
then per 512-wide psum group an epilogue:
    DVE: t = psum + sq2_slice            (sq2 varies along the free dim)
    ACT: o = Ln(t + (1 + sq1[n]))        (per-partition bias), fp16 out
The negate and the fp16->fp32 upcast happen on the HOST during the unshard.

v3 over v2 (v2 = fp8/fp16-out, 1024-wide psum tiles, kk-outer/h-inner):
the PE stream was gap-free at the fp8 roofline, but ntff DMA-packet analysis
showed the ends are HBM-arrival-bound: the three DGE queues (sync/scalar HW,
gpsimd SW) share 16 DMA engines (~340 GB/s), first packet lands ~9.2 us
(after the fixed ~7.5 us NEFF preamble) and v2's pass order needed 1.5 MB
in-flow by pass 8 -> last matmul at ~126 us; the 1024-wide final epilogue +
single-ring drain added ~7.7 us of tail. v3:
  - 512-wide psum groups everywhere (16 banks' worth in 8 [P,512] bufs), in
    4-group COHORTS with kk outer / n inner: cohort 1 of m2=0 needs only
    x1[kk0,n0-3] + x2[m2=0,kk0,h0] (256 KB) to start and adds 128 KB per
    4 passes -- the in-flow requirement grows slower than DMA arrival, so
    the PE never waits on HBM after pass 1 (v2 stalled the whole first
    n-tile on the 1.5 MB wall).
  - x2 DRAM h-major ([MT2, KT8, 2, P, 2, MB]): each (kk, h) 512-col half is
    one contiguous 128 KB read with a 1 KB line per partition.
  - sq2 in fp16 (output error ~1e-4): halves sq2 bytes; m2=0's sq2 is two
    [P, 512] tiles loaded 2nd on the scalar ring, so the first epilogue
    fires right at its psum stop instead of 6 us late (v2 loaded sq2 last,
    eating the entire psum-buffer slack).
  - tail: the final slice's h1 groups run as cohorts {n0-3}, {n4-6}, {n7}:
    the n4-6 epilogues drain behind the last 4 matmul passes, and the final
    group drains in 2x256 chunks with out-DMAs on two different queues.
sq1/sq2 are computed on the host in float64 from the exact inputs (0.01% of
total FLOPs); all N1*N2*D matmul work runs on the NeuronCores.
"""

import time

import numpy as np
import ml_dtypes

import bass_rust
import concourse.bass as bass
import concourse.mybir as mybir
import concourse.tile as tile
from concourse.bass_utils import run_bass_kernel_spmd

# ---------------------------------------------------------------------------
# The pinned walrus rejects instructions carrying more than a small number
# of sem-wait commands ("Too many sync wait commands", CoreV3GenImpl
# setupSyncWait): a drain with 3 waits and a TensorTensor with 3 waits both
# fail; only 1 wait compiles. Post-pass: move excess waits onto NoOp
# instructions inserted immediately before the offender on the same engine
# queue -- waits accumulate across adjacent instructions, so semantics are
# unchanged.
_MAX_WAITS = 1

_split_counter = [0]


def _split_sync_waits(nc, limit=_MAX_WAITS):
    n_split = 0
    for f in nc.m.functions:
        for bb in f.blocks:
            insts = bb.instructions
            out = []
            changed = False
            for inst in insts:
                si = inst.sync_info
                waits = list(si.on_wait) if si and si.on_wait else []
                lim = 1 if inst.engine == mybir.EngineType.SP else limit
                if len(waits) > lim:
                    changed = True
                    n_split += 1
                    excess, keep = waits[:-lim], waits[-lim:]
                    si.on_wait = keep
                    for i in range(0, len(excess), lim):
                        _split_counter[0] += 1
                        nop = mybir.InstNoOp(
                            name=f"I-waitsplit-{_split_counter[0]}",
                            engine=inst.engine,
                            ins=[],
                            outs=[],
                            bass_nofuse=True,
                            sync_info=bass_rust.SyncInfo(
                                on_wait=excess[i:i + lim], on_update=[]
                            ),
                        )
                        out.append(nop)
                out.append(inst)
            if changed:
                bb.instructions = out
    return n_split


N1, N2, D = 8192, 8192, 1024
N_CORES = 8
ROWS = N1 // N_CORES  # 1024 x1 rows per core
P = 128               # SBUF/PSUM partitions
NT = ROWS // P        # 8 n-tiles (output partition tiles) per core
MB = 512              # one fp32 PSUM bank = one psum group width
KT8 = D // 256        # 4 DoubleRow super k-tiles (256 contraction rows each)
MB2 = 2 * MB          # 1024-col m2 slice (2 h-halves)
MT2 = N2 // MB2       # 8 m2 slices
F8 = ml_dtypes.float8_e4m3
F16 = np.float16

_nc_cache = None
last_results = None


def _build_nc(split_waits=True):
    """fp8 e4m3 DoubleRowSwInterleave: 2 contraction rows per PE cell,
    weights pre-interleaved on the host so LDWEIGHTS streams contiguously.

    Operand layout: K = kk*256 + 2*p + j maps contraction row K to
    (partition p, pair-slot j) of super-tile kk on BOTH operands, so
    out[n, m] = sum_{p,j} lhsT[p, j, n] * rhs[p, j, m] is the plain dot
    product. Host arrays are reshaped [D, X] -> [KT8, 128, 2, X] (x1
    additionally SW-interleaved, see kernel()).
    """
    nc = bass.Bass()
    x1t = nc.declare_dram_parameter("x1t", [KT8, P, NT, 2, P], mybir.dt.float8e4, isOutput=False)
    # h-major: each (m2, kk, h) 512-col half is one contiguous 128 KB block
    x2t = nc.declare_dram_parameter("x2t", [MT2, KT8, 2, P, 2, MB], mybir.dt.float8e4, isOutput=False)
    # Host-prebroadcast sq2 ([P, N2], all rows equal), fp16: a device-side
    # partition-broadcast DMA lands on a single engine and takes ~20 us; a
    # plain contiguous read spreads over all 16 DMA engines.
    sq2 = nc.declare_dram_parameter("sq2", [P, N2], mybir.dt.float16, isOutput=False)
    b1 = nc.declare_dram_parameter("b1", [P, NT], mybir.dt.float32, isOutput=False)
    out = nc.declare_dram_parameter("out", [ROWS, N2], mybir.dt.float16, isOutput=True)

    with tile.TileContext(nc) as tc:
        with (
            tc.tile_pool(name="singles", bufs=1) as singles,
            tc.tile_pool(name="x2pool", bufs=16) as x2pool,
            tc.tile_pool(name="psum", bufs=4, space="PSUM") as psumpool,
            # Deep epilogue rings: shallow ones (bufs=4) made ACT(g) wait on
            # the out-DMA transfer of g-4, collapsing the PE pipeline behind
            # DMA latency every ~10 us and re-triggering the pstate ramp.
            tc.tile_pool(name="tpool", bufs=12) as tpool,
            tc.tile_pool(name="opool", bufs=12) as opool,
            tc.tile_pool(name="twide", bufs=6) as twide,
            tc.tile_pool(name="owide", bufs=6) as owide,
        ):
            b1sb = singles.tile([P, NT], mybir.dt.float32)
            x1sb = [
                singles.tile([P, NT, 2, P], mybir.dt.float8e4, tag=f"x1k{kk}", name=f"x1k{kk}")
                for kk in range(KT8)
            ]
            # sq2 SBUF: m2=0 as two [P, MB] half tiles (the h0 half rides
            # 2nd on the scalar ring so the FIRST epilogue isn't sq2-gated);
            # m2>=1 as one [P, MB2] tile each (one 256 KB DMA, 2 KB lines).
            sq2m0h = [
                singles.tile([P, MB], mybir.dt.float16, tag=f"sq2m0h{h}", name=f"sq2m0h{h}")
                for h in range(2)
            ]
            sq2sb = [None] + [
                singles.tile([P, MB2], mybir.dt.float16, tag=f"sq2m{mq}", name=f"sq2m{mq}")
                for mq in range(1, MT2)
            ]

            def new_x2_tile(m2, kk, h):
                return x2pool.tile(
                    [P, 2, MB], mybir.dt.float8e4, tag="x2", name=f"x2_{m2}_{kk}_{h}"
                )

            def load_x2(eng, t, m2, kk, h):
                eng.dma_start(out=t[:], in_=x2t[m2, kk, h])

            HN = NT // 2

            def load_x1_half(eng, kk, hh):
                eng.dma_start(
                    out=x1sb[kk][:, hh * HN:(hh + 1) * HN, :, :],
                    in_=x1t[kk, :, hh * HN:(hh + 1) * HN, :, :],
                )

            def load_sq2(eng, mq):
                eng.dma_start(
                    out=sq2sb[mq][:], in_=sq2[:, mq * MB2:(mq + 1) * MB2]
                )

            # (No PE warmup: the pstate ramp is hidden behind the head-load
            # bandwidth wall, and pre-warming inflates the PE semaphore
            # counter -- the scheduler's coarsened sem-ge targets for the
            # first epilogue adds then sit at the PSUM-backpressure limit.)

            # Head order = dispatch priority = first-use order of the m2=0
            # cohorts (measured: first packet ~1.5 us after dispatch, then
            # ~80-280 GB/s per queue depending on how many queues pull).
            # Every load lands just before its first consuming pass; no
            # non-critical byte rides ahead of a critical one.
            # Head schedule. Measured reality: the 3 DGE queues share 16 DMA
            # engines (~0.25-0.30 MB/us aggregate in the head window), and
            # the tile scheduler COARSENS a matmul's DMA-completion waits to
            # a later ring count (a pass that needs the ring's 2nd tile can
            # end up waiting its ~5th), so fine-grained just-in-time
            # ordering is defeated: the PE stutters, and every multi-us
            # stall re-triggers the ~3 us half-speed pstate ramp. Strategy
            # instead: consolidate ALL head waiting into one pre-start GATE
            # (below) so the PE starts once, ramps once, and runs the 64
            # m2=0 passes stall-free.
            x2cur = {(kk, h): new_x2_tile(0, kk, h) for kk in range(KT8) for h in range(2)}
            # sync: the four h0 rhs tiles + bias, nothing else ahead of the
            # out-DMA stream (keeps the gate's coarsened target ~<=5)
            for kk in range(KT8):
                load_x2(nc.sync, x2cur[(kk, 0)], 0, kk, 0)
            nc.sync.dma_start(out=b1sb[:], in_=b1[:, :])
            # scalar: the x1 A-halves (cohort A), then the h1 rhs tiles
            # (first used at pass 33, ~+7.5 us after start)
            for kk in range(KT8):
                load_x1_half(nc.scalar, kk, 0)
            for kk in range(KT8):
                load_x2(nc.scalar, x2cur[(kk, 1)], 0, kk, 1)
            # gpsimd: m2=0 sq2 halves (first epilogue ~pass 13), x1
            # B-halves (passes 17-29), then the m2=1 prefetch follows.
            nc.gpsimd.dma_start(out=sq2m0h[0][:], in_=sq2[:, 0:MB])
            nc.gpsimd.dma_start(out=sq2m0h[1][:], in_=sq2[:, MB:MB2])
            for kk in range(KT8):
                load_x1_half(nc.gpsimd, kk, 1)

            def sq2_ap(m2, h, lo, hi):
                if m2 == 0:
                    return sq2m0h[h][:, lo:hi]
                return sq2sb[m2][:, h * MB + lo:h * MB + hi]

            def epilogue(ps, m2, n, h, bounds=((0, MB),), dma_engs=None):
                pt, cb = ps
                t = tpool.tile([P, MB], mybir.dt.float32)
                o = opool.tile([P, MB], mybir.dt.float16)
                base = m2 * MB2 + h * MB
                for ci, (lo, hi) in enumerate(bounds):
                    nc.vector.tensor_add(t[:, lo:hi], pt[:, cb + lo:cb + hi], sq2_ap(m2, h, lo, hi))
                    nc.scalar.activation(
                        out=o[:, lo:hi],
                        in_=t[:, lo:hi],
                        func=mybir.ActivationFunctionType.Ln,
                        bias=b1sb[:, n:n + 1],
                        scale=1.0,
                    )
                    eng = dma_engs[ci] if dma_engs else nc.sync
                    eng.dma_start(
                        out=out[n * P:(n + 1) * P, base + lo:base + hi],
                        in_=o[:, lo:hi],
                    )

            for m2 in range(MT2):
                x2m = x2cur
                x2nxt = None
                if m2 + 1 < MT2:
                    # m2+1 prefetch on gpsimd SWDGE (own DMASW sem lanes --
                    # a HWDGE ring would couple it to out-DMA lane turnover)
                    x2nxt = {}
                    for kk in range(KT8):
                        for h in range(2):
                            tn = new_x2_tile(m2 + 1, kk, h)
                            load_x2(nc.gpsimd, tn, m2 + 1, kk, h)
                            x2nxt[(kk, h)] = tn
                            if kk == 1 and h == 1:
                                load_sq2(nc.gpsimd, m2 + 1)
                if 0 < m2 < MT2 - 1:
                    # steady state: v2-style 1024-wide groups (half the
                    # epilogue instruction count -- 512-wide everywhere
                    # pushed the ACT engine to 110 us, co-bottlenecking
                    # with the PE's 113); kk outer / h inner so both
                    # 512-col passes stream against stationary weights.
                    for n in range(NT):
                        psw = psumpool.tile(
                            [P, MB2], mybir.dt.float32,
                            tag="ps", name=f"psw_{m2}_{n}",
                        )
                        for kk in range(KT8):
                            for h in range(2):
                                nc.tensor.matmul(
                                    psw[:, h * MB:(h + 1) * MB],
                                    lhsT=x1sb[kk][:, n, :, :],
                                    rhs=x2m[(kk, h)][:],
                                    start=(kk == 0),
                                    stop=(kk == KT8 - 1),
                                    skip_group_check=True,
                                    perf_mode=mybir.MatmulPerfMode.DoubleRowSwInterleave,
                                )
                        tw = twide.tile([P, MB2], mybir.dt.float32, tag="tw", name=f"tw_{m2}_{n}")
                        ow = owide.tile([P, MB2], mybir.dt.float16, tag="ow", name=f"ow_{m2}_{n}")
                        nc.vector.tensor_add(tw[:], psw[:], sq2sb[m2][:])
                        nc.scalar.activation(
                            out=ow[:],
                            in_=tw[:],
                            func=mybir.ActivationFunctionType.Ln,
                            bias=b1sb[:, n:n + 1],
                            scale=1.0,
                        )
                        nc.sync.dma_start(
                            out=out[n * P:(n + 1) * P, m2 * MB2:(m2 + 1) * MB2],
                            in_=ow[:],
                        )
                    if x2nxt is not None:
                        x2cur = x2nxt
                    continue
                last_slice_h1 = (m2 == MT2 - 1)
                for h in range(2):
                    if last_slice_h1 and h == 1:
                        # drain-friendly cohorts: the n4-6 epilogues overlap
                        # the final 4 passes; the last group drains alone
                        cohorts = [(0, 1, 2, 3), (4, 5, 6), (7,)]
                    else:
                        cohorts = [(0, 1, 2, 3), (4, 5, 6, 7)]
                    for ns in cohorts:
                        # PSUM slots are 2-bank granular: allocate [P, MB2]
                        # tiles and give each group one independent 512-wide
                        # half (separate accumulation region, region-granular
                        # deps) -- 4 tiles in the pool = 8 groups in flight.
                        ptiles = [
                            psumpool.tile(
                                [P, MB2], mybir.dt.float32,
                                tag="ps", name=f"ps_{m2}_{h}_{ns[0]}_{i}",
                            )
                            for i in range((len(ns) + 1) // 2)
                        ]
                        pss = {
                            n: (ptiles[i // 2], (i % 2) * MB)
                            for i, n in enumerate(ns)
                        }
                        if m2 == 0 and h == 0 and ns[0] == 0:
                            # GATE: a 16-column matmul reading the LAST
                            # h0/x1A head tiles into a dead psum region
                            # (immediately re-zeroed by the kk0 start pass).
                            # It delays the PE start until the whole h0
                            # working set is resident, so the 64-pass m2=0
                            # stream runs with ONE ramp and no mid-stream
                            # stalls (starting at first-tile-arrival gained
                            # 3 us of start but lost 5+ to stutter+re-ramp).
                            nc.tensor.matmul(
                                ptiles[0][:, 0:16],
                                lhsT=x1sb[3][:, 3, :, :],
                                rhs=x2cur[(3, 0)][:, :, 0:16],
                                start=True,
                                stop=True,
                                skip_group_check=True,
                                perf_mode=mybir.MatmulPerfMode.DoubleRowSwInterleave,
                            )
                        # kk outer / n inner: each new 128 KB x2 half (and
                        # 128 KB x1 half) unlocks 4 more passes
                        for kk in range(KT8):
                            for n in ns:
                                pt, cb = pss[n]
                                nc.tensor.matmul(
                                    pt[:, cb:cb + MB],
                                    lhsT=x1sb[kk][:, n, :, :],
                                    rhs=x2m[(kk, h)][:],
                                    start=(kk == 0),
                                    stop=(kk == KT8 - 1),
                                    skip_group_check=True,
                                    perf_mode=mybir.MatmulPerfMode.DoubleRowSwInterleave,
                                )
                        for n in ns:
                            final = last_slice_h1 and h == 1 and n == NT - 1
                            if final:
                                # chunked drain; both chunk DMAs on HWDGE
                                # queues -- a gpsimd (SWDGE) out-DMA here
                                # put its ~7 us queue drain on the tail
                                QB = MB // 2
                                epilogue(
                                    pss[n], m2, n, h,
                                    bounds=((0, QB), (QB, MB)),
                                    dma_engs=[nc.sync, nc.scalar],
                                )
                            else:
                                epilogue(pss[n], m2, n, h)
                if x2nxt is not None:
                    x2cur = x2nxt
    if split_waits:
        _split_sync_waits(nc)
    return nc


def kernel(x1, x2, _trace=False):
    global _nc_cache, last_results
    x1f = np.asarray(x1, dtype=np.float32)
    x2f = np.asarray(x2, dtype=np.float32)
    assert x1f.shape == (N1, D) and x2f.shape == (N2, D)

    a8 = (-2.0 * x1f).astype(F8)                # [N1, D] fp8(-2 x1)
    x2_8 = x2f.astype(F8)                       # [N2, D]
    x1ts = np.ascontiguousarray(a8.T).reshape(KT8, P, 2, N1)
    # [KT8, P, 2, N2] -> h-major [MT2, KT8, 2, P, 2, MB] (one contiguous
    # 128 KB block per (m2, kk, h) device tile)
    x2t = np.ascontiguousarray(
        x2_8.T.reshape(KT8, P, 2, MT2, 2, MB).transpose(3, 0, 4, 1, 2, 5)
    )
    # SwInterleave weight layout: per 128-column block, pairs (j=0, j=1)
    # interleaved per column with columns reversed:
    # flat[q] with q = 2*(127-c) + j  <->  logical[j, c]
    g = x1ts.reshape(KT8, P, 2, N1 // P, P)           # [kk, p, j, nblk, c]
    g = g[:, :, :, :, ::-1].transpose(0, 1, 3, 4, 2)  # [kk, p, nblk, c~, j]
    x1ts = np.ascontiguousarray(g).reshape(KT8, P, N1 // P, 2, P)

    sq1 = (x1f.astype(np.float64) ** 2).sum(axis=-1)
    sq2 = (x2f.astype(np.float64) ** 2).sum(axis=-1)
    bias1 = (1.0 + sq1).astype(np.float32)        # [N1]
    # host-side partition broadcast (see sq2 dram param comment), fp16
    sq2_bc = np.ascontiguousarray(
        np.broadcast_to(sq2.astype(np.float16).reshape(1, N2), (P, N2))
    )

    in_maps = []
    for c in range(N_CORES):
        r0, r1 = c * ROWS, (c + 1) * ROWS
        in_maps.append({
            "x1t": np.ascontiguousarray(x1ts[:, :, c * NT:(c + 1) * NT]),
            "x2t": x2t,
            "sq2": sq2_bc,
            # b1[p, n] = 1 + sq1[r0 + n*128 + p]
            "b1": np.ascontiguousarray(bias1[r0:r1].reshape(NT, P).T),
        })

    if _nc_cache is None:
        _nc_cache = _build_nc()
    res = None
    for attempt in range(3):
        try:
            res = run_bass_kernel_spmd(
                _nc_cache, in_maps, core_ids=list(range(N_CORES)), trace=_trace
            )
            break
        except Exception:
            if attempt == 2:
                raise
            time.sleep(5.0)
    last_results = res
    # Device computes +log1p(d2) in fp16; the sign flip and f32 upcast are
    # part of the host-side unshard.
    full = np.concatenate([res.results[c]["out"] for c in range(N_CORES)], axis=0)
    return -full.astype(np.float32)


# revision 18
# speedup vs baseline: 1.0097x; 1.0097x over previous
"""Pairwise squared-euclidean-distance kernel (-log1p(max(d2,0))) for 8 trn2 cores.

Strategy (sharding_hint): shard x1 rows across the 8 NeuronCores (1024 rows
each); replicate x2. Each core computes a [1024, 8192] slab of the output:

    out[n, m] = -log1p(sq1[n] + sq2[m] - 2 * x1[n] . x2[m])

Device work per core: a [1024 x 1024] @ [1024 x 8192] matmul into PSUM
(psum = -2 * cross, the -2 baked into the lhsT operand on the host; fp8 e4m3
operands, DoubleRowSwInterleave so each 512-col pass covers 256 contraction
rows -- the HW-max rate of 1 moving column/cycle at 2.4 GHz, ~216 ns/pass),
then per 512-wide psum group an epilogue:
    DVE: t = psum + sq2_slice            (sq2 varies along the free dim)
    ACT: o = Ln(t + (1 + sq1[n]))        (per-partition bias), fp16 out
The negate and the fp16->fp32 upcast happen on the HOST during the unshard.

v3 over v2 (v2 = fp8/fp16-out, 1024-wide psum tiles, kk-outer/h-inner):
the PE stream was gap-free at the fp8 roofline, but ntff DMA-packet analysis
showed the ends are HBM-arrival-bound: the three DGE queues (sync/scalar HW,
gpsimd SW) share 16 DMA engines (~340 GB/s), first packet lands ~9.2 us
(after the fixed ~7.5 us NEFF preamble) and v2's pass order needed 1.5 MB
in-flow by pass 8 -> last matmul at ~126 us; the 1024-wide final epilogue +
single-ring drain added ~7.7 us of tail. v3:
  - 512-wide psum groups everywhere (16 banks' worth in 8 [P,512] bufs), in
    4-group COHORTS with kk outer / n inner: cohort 1 of m2=0 needs only
    x1[kk0,n0-3] + x2[m2=0,kk0,h0] (256 KB) to start and adds 128 KB per
    4 passes -- the in-flow requirement grows slower than DMA arrival, so
    the PE never waits on HBM after pass 1 (v2 stalled the whole first
    n-tile on the 1.5 MB wall).
  - x2 DRAM h-major ([MT2, KT8, 2, P, 2, MB]): each (kk, h) 512-col half is
    one contiguous 128 KB read with a 1 KB line per partition.
  - sq2 in fp16 (output error ~1e-4): halves sq2 bytes; m2=0's sq2 is two
    [P, 512] tiles loaded 2nd on the scalar ring, so the first epilogue
    fires right at its psum stop instead of 6 us late (v2 loaded sq2 last,
    eating the entire psum-buffer slack).
  - tail: the final slice's h1 groups run as cohorts {n0-3}, {n4-6}, {n7}:
    the n4-6 epilogues drain behind the last 4 matmul passes, and the final
    group drains in 2x256 chunks with out-DMAs on two different queues.
sq1/sq2 are computed on the host in float64 from the exact inputs (0.01% of
total FLOPs); all N1*N2*D matmul work runs on the NeuronCores.
"""

import time

import numpy as np
import ml_dtypes

import bass_rust
import concourse.bass as bass
import concourse.mybir as mybir
import concourse.tile as tile
from concourse.bass_utils import run_bass_kernel_spmd

# ---------------------------------------------------------------------------
# The pinned walrus rejects instructions carrying more than a small number
# of sem-wait commands ("Too many sync wait commands", CoreV3GenImpl
# setupSyncWait): a drain with 3 waits and a TensorTensor with 3 waits both
# fail; only 1 wait compiles. Post-pass: move excess waits onto NoOp
# instructions inserted immediately before the offender on the same engine
# queue -- waits accumulate across adjacent instructions, so semantics are
# unchanged.
_MAX_WAITS = 1

_split_counter = [0]


def _split_sync_waits(nc, limit=_MAX_WAITS):
    n_split = 0
    for f in nc.m.functions:
        for bb in f.blocks:
            insts = bb.instructions
            out = []
            changed = False
            for inst in insts:
                si = inst.sync_info
                waits = list(si.on_wait) if si and si.on_wait else []
                lim = 1 if inst.engine == mybir.EngineType.SP else limit
                if len(waits) > lim:
                    changed = True
                    n_split += 1
                    excess, keep = waits[:-lim], waits[-lim:]
                    si.on_wait = keep
                    for i in range(0, len(excess), lim):
                        _split_counter[0] += 1
                        nop = mybir.InstNoOp(
                            name=f"I-waitsplit-{_split_counter[0]}",
                            engine=inst.engine,
                            ins=[],
                            outs=[],
                            bass_nofuse=True,
                            sync_info=bass_rust.SyncInfo(
                                on_wait=excess[i:i + lim], on_update=[]
                            ),
                        )
                        out.append(nop)
                out.append(inst)
            if changed:
                bb.instructions = out
    return n_split


N1, N2, D = 8192, 8192, 1024
N_CORES = 8
ROWS = N1 // N_CORES  # 1024 x1 rows per core
P = 128               # SBUF/PSUM partitions
NT = ROWS // P        # 8 n-tiles (output partition tiles) per core
MB = 512              # one fp32 PSUM bank = one psum group width
KT8 = D // 256        # 4 DoubleRow super k-tiles (256 contraction rows each)
MB2 = 2 * MB          # 1024-col m2 slice (2 h-halves)
MT2 = N2 // MB2       # 8 m2 slices
F8 = ml_dtypes.float8_e4m3
F16 = np.float16

_nc_cache = None
last_results = None


def _build_nc(split_waits=True):
    """fp8 e4m3 DoubleRowSwInterleave: 2 contraction rows per PE cell,
    weights pre-interleaved on the host so LDWEIGHTS streams contiguously.

    Operand layout: K = kk*256 + 2*p + j maps contraction row K to
    (partition p, pair-slot j) of super-tile kk on BOTH operands, so
    out[n, m] = sum_{p,j} lhsT[p, j, n] * rhs[p, j, m] is the plain dot
    product. Host arrays are reshaped [D, X] -> [KT8, 128, 2, X] (x1
    additionally SW-interleaved, see kernel()).
    """
    nc = bass.Bass()
    x1t = nc.declare_dram_parameter("x1t", [KT8, P, NT, 2, P], mybir.dt.float8e4, isOutput=False)
    # h-major: each (m2, kk, h) 512-col half is one contiguous 128 KB block
    x2t = nc.declare_dram_parameter("x2t", [MT2, KT8, 2, P, 2, MB], mybir.dt.float8e4, isOutput=False)
    # Host-prebroadcast sq2 ([P, N2], all rows equal), fp16: a device-side
    # partition-broadcast DMA lands on a single engine and takes ~20 us; a
    # plain contiguous read spreads over all 16 DMA engines.
    sq2 = nc.declare_dram_parameter("sq2", [P, N2], mybir.dt.float16, isOutput=False)
    b1 = nc.declare_dram_parameter("b1", [P, NT], mybir.dt.float32, isOutput=False)
    out = nc.declare_dram_parameter("out", [ROWS, N2], mybir.dt.float16, isOutput=True)

    with tile.TileContext(nc) as tc:
        with (
            tc.tile_pool(name="singles", bufs=1) as singles,
            tc.tile_pool(name="x2pool", bufs=16) as x2pool,
            tc.tile_pool(name="psum", bufs=4, space="PSUM") as psumpool,
            # Deep epilogue rings: shallow ones (bufs=4) made ACT(g) wait on
            # the out-DMA transfer of g-4, collapsing the PE pipeline behind
            # DMA latency every ~10 us and re-triggering the pstate ramp.
            tc.tile_pool(name="tpool", bufs=12) as tpool,
            tc.tile_pool(name="opool", bufs=12) as opool,
            tc.tile_pool(name="twide", bufs=6) as twide,
            tc.tile_pool(name="owide", bufs=6) as owide,
        ):
            b1sb = singles.tile([P, NT], mybir.dt.float32)
            x1sb = [
                singles.tile([P, NT, 2, P], mybir.dt.float8e4, tag=f"x1k{kk}", name=f"x1k{kk}")
                for kk in range(KT8)
            ]
            # sq2 SBUF: m2=0 as two [P, MB] half tiles (the h0 half rides
            # 2nd on the scalar ring so the FIRST epilogue isn't sq2-gated);
            # m2>=1 as one [P, MB2] tile each (one 256 KB DMA, 2 KB lines).
            sq2m0h = [
                singles.tile([P, MB], mybir.dt.float16, tag=f"sq2m0h{h}", name=f"sq2m0h{h}")
                for h in range(2)
            ]
            sq2sb = [None] + [
                singles.tile([P, MB2], mybir.dt.float16, tag=f"sq2m{mq}", name=f"sq2m{mq}")
                for mq in range(1, MT2)
            ]

            def new_x2_tile(m2, kk, h):
                return x2pool.tile(
                    [P, 2, MB], mybir.dt.float8e4, tag="x2", name=f"x2_{m2}_{kk}_{h}"
                )

            def load_x2(eng, t, m2, kk, h):
                eng.dma_start(out=t[:], in_=x2t[m2, kk, h])

            HN = NT // 2

            def load_x1_half(eng, kk, hh):
                eng.dma_start(
                    out=x1sb[kk][:, hh * HN:(hh + 1) * HN, :, :],
                    in_=x1t[kk, :, hh * HN:(hh + 1) * HN, :, :],
                )

            def load_sq2(eng, mq):
                eng.dma_start(
                    out=sq2sb[mq][:], in_=sq2[:, mq * MB2:(mq + 1) * MB2]
                )

            # (No PE warmup: the pstate ramp is hidden behind the head-load
            # bandwidth wall, and pre-warming inflates the PE semaphore
            # counter -- the scheduler's coarsened sem-ge targets for the
            # first epilogue adds then sit at the PSUM-backpressure limit.)

            # Head order = dispatch priority = first-use order of the m2=0
            # cohorts (measured: first packet ~1.5 us after dispatch, then
            # ~80-280 GB/s per queue depending on how many queues pull).
            # Every load lands just before its first consuming pass; no
            # non-critical byte rides ahead of a critical one.
            # Head schedule. Measured reality: the 3 DGE queues share 16 DMA
            # engines (~0.25-0.30 MB/us aggregate in the head window), and
            # the tile scheduler COARSENS a matmul's DMA-completion waits to
            # a later ring count (a pass that needs the ring's 2nd tile can
            # end up waiting its ~5th), so fine-grained just-in-time
            # ordering is defeated: the PE stutters, and every multi-us
            # stall re-triggers the ~3 us half-speed pstate ramp. Strategy
            # instead: consolidate ALL head waiting into one pre-start GATE
            # (below) so the PE starts once, ramps once, and runs the 64
            # m2=0 passes stall-free.
            x2cur = {(kk, h): new_x2_tile(0, kk, h) for kk in range(KT8) for h in range(2)}
            # sync: the four h0 rhs tiles + bias, nothing else ahead of the
            # out-DMA stream (keeps the gate's coarsened target ~<=5)
            for kk in range(KT8):
                load_x2(nc.sync, x2cur[(kk, 0)], 0, kk, 0)
            nc.sync.dma_start(out=b1sb[:], in_=b1[:, :])
            # scalar: the x1 A-halves (cohort A), then the h1 rhs tiles
            # (first used at pass 33, ~+7.5 us after start)
            for kk in range(KT8):
                load_x1_half(nc.scalar, kk, 0)
            for kk in range(KT8):
                load_x2(nc.scalar, x2cur[(kk, 1)], 0, kk, 1)
            # gpsimd: m2=0 sq2 halves (first epilogue ~pass 13), x1
            # B-halves (passes 17-29), then the m2=1 prefetch follows.
            nc.gpsimd.dma_start(out=sq2m0h[0][:], in_=sq2[:, 0:MB])
            nc.gpsimd.dma_start(out=sq2m0h[1][:], in_=sq2[:, MB:MB2])
            for kk in range(KT8):
                load_x1_half(nc.gpsimd, kk, 1)

            def sq2_ap(m2, h, lo, hi):
                if m2 == 0:
                    return sq2m0h[h][:, lo:hi]
                return sq2sb[m2][:, h * MB + lo:h * MB + hi]

            def epilogue(ps, m2, n, h, bounds=((0, MB),), dma_engs=None):
                pt, cb = ps
                t = tpool.tile([P, MB], mybir.dt.float32)
                o = opool.tile([P, MB], mybir.dt.float16)
                base = m2 * MB2 + h * MB
                for ci, (lo, hi) in enumerate(bounds):
                    nc.vector.tensor_add(t[:, lo:hi], pt[:, cb + lo:cb + hi], sq2_ap(m2, h, lo, hi))
                    nc.scalar.activation(
                        out=o[:, lo:hi],
                        in_=t[:, lo:hi],
                        func=mybir.ActivationFunctionType.Ln,
                        bias=b1sb[:, n:n + 1],
                        scale=1.0,
                    )
                    eng = dma_engs[ci] if dma_engs else nc.sync
                    eng.dma_start(
                        out=out[n * P:(n + 1) * P, base + lo:base + hi],
                        in_=o[:, lo:hi],
                    )

            for m2 in range(MT2):
                x2m = x2cur
                x2nxt = None
                if m2 + 1 < MT2:
                    # m2+1 prefetch on gpsimd SWDGE (own DMASW sem lanes --
                    # a HWDGE ring would couple it to out-DMA lane turnover)
                    x2nxt = {}
                    for kk in range(KT8):
                        for h in range(2):
                            tn = new_x2_tile(m2 + 1, kk, h)
                            load_x2(nc.gpsimd, tn, m2 + 1, kk, h)
                            x2nxt[(kk, h)] = tn
                            if kk == 1 and h == 1:
                                load_sq2(nc.gpsimd, m2 + 1)
                if 0 < m2 < MT2 - 1:
                    # steady state: v2-style 1024-wide groups (half the
                    # epilogue instruction count -- 512-wide everywhere
                    # pushed the ACT engine to 110 us, co-bottlenecking
                    # with the PE's 113); kk outer / h inner so both
                    # 512-col passes stream against stationary weights.
                    for n in range(NT):
                        psw = psumpool.tile(
                            [P, MB2], mybir.dt.float32,
                            tag="ps", name=f"psw_{m2}_{n}",
                        )
                        for kk in range(KT8):
                            for h in range(2):
                                nc.tensor.matmul(
                                    psw[:, h * MB:(h + 1) * MB],
                                    lhsT=x1sb[kk][:, n, :, :],
                                    rhs=x2m[(kk, h)][:],
                                    start=(kk == 0),
                                    stop=(kk == KT8 - 1),
                                    skip_group_check=True,
                                    perf_mode=mybir.MatmulPerfMode.DoubleRowSwInterleave,
                                )
                        tw = twide.tile([P, MB2], mybir.dt.float32, tag="tw", name=f"tw_{m2}_{n}")
                        ow = owide.tile([P, MB2], mybir.dt.float16, tag="ow", name=f"ow_{m2}_{n}")
                        nc.vector.tensor_add(tw[:], psw[:], sq2sb[m2][:])
                        nc.scalar.activation(
                            out=ow[:],
                            in_=tw[:],
                            func=mybir.ActivationFunctionType.Ln,
                            bias=b1sb[:, n:n + 1],
                            scale=1.0,
                        )
                        nc.sync.dma_start(
                            out=out[n * P:(n + 1) * P, m2 * MB2:(m2 + 1) * MB2],
                            in_=ow[:],
                        )
                    if x2nxt is not None:
                        x2cur = x2nxt
                    continue
                last_slice_h1 = (m2 == MT2 - 1)
                for h in range(2):
                    if last_slice_h1 and h == 1:
                        # drain-friendly cohorts: the n4-6 epilogues overlap
                        # the final 4 passes; the last group drains alone
                        cohorts = [(0, 1, 2, 3), (4, 5, 6), (7,)]
                    else:
                        cohorts = [(0, 1, 2, 3), (4, 5, 6, 7)]
                    for ns in cohorts:
                        # PSUM slots are 2-bank granular: allocate [P, MB2]
                        # tiles and give each group one independent 512-wide
                        # half (separate accumulation region, region-granular
                        # deps) -- 4 tiles in the pool = 8 groups in flight.
                        ptiles = [
                            psumpool.tile(
                                [P, MB2], mybir.dt.float32,
                                tag="ps", name=f"ps_{m2}_{h}_{ns[0]}_{i}",
                            )
                            for i in range((len(ns) + 1) // 2)
                        ]
                        pss = {
                            n: (ptiles[i // 2], (i % 2) * MB)
                            for i, n in enumerate(ns)
                        }
                        if m2 == 0 and h == 0 and ns[0] == 0:
                            # GATE: a 16-column matmul reading the LAST
                            # h0/x1A head tiles into a dead psum region
                            # (immediately re-zeroed by the kk0 start pass).
                            # It delays the PE start until the whole h0
                            # working set is resident, so the 64-pass m2=0
                            # stream runs with ONE ramp and no mid-stream
                            # stalls (starting at first-tile-arrival gained
                            # 3 us of start but lost 5+ to stutter+re-ramp).
                            # Gate on the 3rd wave (ring delivery is ~2 us
                            # per 128 KB tile): passes 1-12's operands are
                            # then resident, and the 4th wave (x2_30h0 ~17.8,
                            # x1k3A ~17.8) lands before pass 13's natural
                            # time (~20.4) even if its wait is coarsened.
                            nc.tensor.matmul(
                                ptiles[0][:, 0:16],
                                lhsT=x1sb[2][:, 2, :, :],
                                rhs=x2cur[(2, 0)][:, :, 0:16],
                                start=True,
                                stop=True,
                                skip_group_check=True,
                                perf_mode=mybir.MatmulPerfMode.DoubleRowSwInterleave,
                            )
                        # kk outer / n inner: each new 128 KB x2 half (and
                        # 128 KB x1 half) unlocks 4 more passes
                        for kk in range(KT8):
                            for n in ns:
                                pt, cb = pss[n]
                                nc.tensor.matmul(
                                    pt[:, cb:cb + MB],
                                    lhsT=x1sb[kk][:, n, :, :],
                                    rhs=x2m[(kk, h)][:],
                                    start=(kk == 0),
                                    stop=(kk == KT8 - 1),
                                    skip_group_check=True,
                                    perf_mode=mybir.MatmulPerfMode.DoubleRowSwInterleave,
                                )
                        for n in ns:
                            final = last_slice_h1 and h == 1 and n == NT - 1
                            if final:
                                # chunked drain; both chunk DMAs on HWDGE
                                # queues -- a gpsimd (SWDGE) out-DMA here
                                # put its ~7 us queue drain on the tail
                                QB = MB // 2
                                epilogue(
                                    pss[n], m2, n, h,
                                    bounds=((0, QB), (QB, MB)),
                                    dma_engs=[nc.sync, nc.scalar],
                                )
                            else:
                                epilogue(pss[n], m2, n, h)
                if x2nxt is not None:
                    x2cur = x2nxt
    if split_waits:
        _split_sync_waits(nc)
    return nc


def kernel(x1, x2, _trace=False):
    global _nc_cache, last_results
    x1f = np.asarray(x1, dtype=np.float32)
    x2f = np.asarray(x2, dtype=np.float32)
    assert x1f.shape == (N1, D) and x2f.shape == (N2, D)

    a8 = (-2.0 * x1f).astype(F8)                # [N1, D] fp8(-2 x1)
    x2_8 = x2f.astype(F8)                       # [N2, D]
    x1ts = np.ascontiguousarray(a8.T).reshape(KT8, P, 2, N1)
    # [KT8, P, 2, N2] -> h-major [MT2, KT8, 2, P, 2, MB] (one contiguous
    # 128 KB block per (m2, kk, h) device tile)
    x2t = np.ascontiguousarray(
        x2_8.T.reshape(KT8, P, 2, MT2, 2, MB).transpose(3, 0, 4, 1, 2, 5)
    )
    # SwInterleave weight layout: per 128-column block, pairs (j=0, j=1)
    # interleaved per column with columns reversed:
    # flat[q] with q = 2*(127-c) + j  <->  logical[j, c]
    g = x1ts.reshape(KT8, P, 2, N1 // P, P)           # [kk, p, j, nblk, c]
    g = g[:, :, :, :, ::-1].transpose(0, 1, 3, 4, 2)  # [kk, p, nblk, c~, j]
    x1ts = np.ascontiguousarray(g).reshape(KT8, P, N1 // P, 2, P)

    sq1 = (x1f.astype(np.float64) ** 2).sum(axis=-1)
    sq2 = (x2f.astype(np.float64) ** 2).sum(axis=-1)
    bias1 = (1.0 + sq1).astype(np.float32)        # [N1]
    # host-side partition broadcast (see sq2 dram param comment), fp16
    sq2_bc = np.ascontiguousarray(
        np.broadcast_to(sq2.astype(np.float16).reshape(1, N2), (P, N2))
    )

    in_maps = []
    for c in range(N_CORES):
        r0, r1 = c * ROWS, (c + 1) * ROWS
        in_maps.append({
            "x1t": np.ascontiguousarray(x1ts[:, :, c * NT:(c + 1) * NT]),
            "x2t": x2t,
            "sq2": sq2_bc,
            # b1[p, n] = 1 + sq1[r0 + n*128 + p]
            "b1": np.ascontiguousarray(bias1[r0:r1].reshape(NT, P).T),
        })

    if _nc_cache is None:
        _nc_cache = _build_nc()
    res = None
    for attempt in range(3):
        try:
            res = run_bass_kernel_spmd(
                _nc_cache, in_maps, core_ids=list(range(N_CORES)), trace=_trace
            )
            break
        except Exception:
            if attempt == 2:
                raise
            time.sleep(5.0)
    last_results = res
    # Device computes +log1p(d2) in fp16; the sign flip and f32 upcast are
    # part of the host-side unshard.
    full = np.concatenate([res.results[c]["out"] for c in range(N_CORES)], axis=0)
    return -full.astype(np.float32)


# revision 20
# speedup vs baseline: 1.0282x; 1.0183x over previous
"""Pairwise squared-euclidean-distance kernel (-log1p(max(d2,0))) for 8 trn2 cores.

Strategy (sharding_hint): shard x1 rows across the 8 NeuronCores (1024 rows
each); replicate x2. Each core computes a [1024, 8192] slab of the output:

    out[n, m] = -log1p(sq1[n] + sq2[m] - 2 * x1[n] . x2[m])

Device work per core: a [1024 x 1024] @ [1024 x 8192] matmul into PSUM
(psum = -2 * cross, the -2 baked into the lhsT operand on the host; fp8 e4m3
operands, DoubleRowSwInterleave so each 512-col pass covers 256 contraction
rows -- the HW-max rate of 1 moving column/cycle at 2.4 GHz, ~216 ns/pass),
then per 512-wide psum group an epilogue:
    DVE: t = psum + sq2_slice            (sq2 varies along the free dim)
    ACT: o = Ln(t + (1 + sq1[n]))        (per-partition bias), fp16 out
The negate and the fp16->fp32 upcast happen on the HOST during the unshard.

v3 over v2 (v2 = fp8/fp16-out, 1024-wide psum tiles, kk-outer/h-inner):
the PE stream was gap-free at the fp8 roofline, but ntff DMA-packet analysis
showed the ends are HBM-arrival-bound: the three DGE queues (sync/scalar HW,
gpsimd SW) share 16 DMA engines (~340 GB/s), first packet lands ~9.2 us
(after the fixed ~7.5 us NEFF preamble) and v2's pass order needed 1.5 MB
in-flow by pass 8 -> last matmul at ~126 us; the 1024-wide final epilogue +
single-ring drain added ~7.7 us of tail. v3:
  - 512-wide psum groups everywhere (16 banks' worth in 8 [P,512] bufs), in
    4-group COHORTS with kk outer / n inner: cohort 1 of m2=0 needs only
    x1[kk0,n0-3] + x2[m2=0,kk0,h0] (256 KB) to start and adds 128 KB per
    4 passes -- the in-flow requirement grows slower than DMA arrival, so
    the PE never waits on HBM after pass 1 (v2 stalled the whole first
    n-tile on the 1.5 MB wall).
  - x2 DRAM h-major ([MT2, KT8, 2, P, 2, MB]): each (kk, h) 512-col half is
    one contiguous 128 KB read with a 1 KB line per partition.
  - sq2 in fp16 (output error ~1e-4): halves sq2 bytes; m2=0's sq2 is two
    [P, 512] tiles loaded 2nd on the scalar ring, so the first epilogue
    fires right at its psum stop instead of 6 us late (v2 loaded sq2 last,
    eating the entire psum-buffer slack).
  - tail: the final slice's h1 groups run as cohorts {n0-3}, {n4-6}, {n7}:
    the n4-6 epilogues drain behind the last 4 matmul passes, and the final
    group drains in 2x256 chunks with out-DMAs on two different queues.
sq1/sq2 are computed on the host in float64 from the exact inputs (0.01% of
total FLOPs); all N1*N2*D matmul work runs on the NeuronCores.
"""

import time

import numpy as np
import ml_dtypes

import bass_rust
import concourse.bass as bass
import concourse.mybir as mybir
import concourse.tile as tile
from concourse.bass_utils import run_bass_kernel_spmd

# ---------------------------------------------------------------------------
# The pinned walrus rejects instructions carrying more than a small number
# of sem-wait commands ("Too many sync wait commands", CoreV3GenImpl
# setupSyncWait): a drain with 3 waits and a TensorTensor with 3 waits both
# fail; only 1 wait compiles. Post-pass: move excess waits onto NoOp
# instructions inserted immediately before the offender on the same engine
# queue -- waits accumulate across adjacent instructions, so semantics are
# unchanged.
_MAX_WAITS = 1

_split_counter = [0]


def _split_sync_waits(nc, limit=_MAX_WAITS):
    n_split = 0
    for f in nc.m.functions:
        for bb in f.blocks:
            insts = bb.instructions
            out = []
            changed = False
            for inst in insts:
                si = inst.sync_info
                waits = list(si.on_wait) if si and si.on_wait else []
                lim = 1 if inst.engine == mybir.EngineType.SP else limit
                if len(waits) > lim:
                    changed = True
                    n_split += 1
                    excess, keep = waits[:-lim], waits[-lim:]
                    si.on_wait = keep
                    for i in range(0, len(excess), lim):
                        _split_counter[0] += 1
                        nop = mybir.InstNoOp(
                            name=f"I-waitsplit-{_split_counter[0]}",
                            engine=inst.engine,
                            ins=[],
                            outs=[],
                            bass_nofuse=True,
                            sync_info=bass_rust.SyncInfo(
                                on_wait=excess[i:i + lim], on_update=[]
                            ),
                        )
                        out.append(nop)
                out.append(inst)
            if changed:
                bb.instructions = out
    return n_split


N1, N2, D = 8192, 8192, 1024
N_CORES = 8
ROWS = N1 // N_CORES  # 1024 x1 rows per core
P = 128               # SBUF/PSUM partitions
NT = ROWS // P        # 8 n-tiles (output partition tiles) per core
MB = 512              # one fp32 PSUM bank = one psum group width
KT8 = D // 256        # 4 DoubleRow super k-tiles (256 contraction rows each)
MB2 = 2 * MB          # 1024-col m2 slice (2 h-halves)
MT2 = N2 // MB2       # 8 m2 slices
F8 = ml_dtypes.float8_e4m3
F16 = np.float16

_nc_cache = None
last_results = None


def _build_nc(split_waits=True):
    """fp8 e4m3 DoubleRowSwInterleave: 2 contraction rows per PE cell,
    weights pre-interleaved on the host so LDWEIGHTS streams contiguously.

    Operand layout: K = kk*256 + 2*p + j maps contraction row K to
    (partition p, pair-slot j) of super-tile kk on BOTH operands, so
    out[n, m] = sum_{p,j} lhsT[p, j, n] * rhs[p, j, m] is the plain dot
    product. Host arrays are reshaped [D, X] -> [KT8, 128, 2, X] (x1
    additionally SW-interleaved, see kernel()).
    """
    nc = bass.Bass()
    x1t = nc.declare_dram_parameter("x1t", [KT8, P, NT, 2, P], mybir.dt.float8e4, isOutput=False)
    # h-major: each (m2, kk, h) 512-col half is one contiguous 128 KB block
    x2t = nc.declare_dram_parameter("x2t", [MT2, KT8, 2, P, 2, MB], mybir.dt.float8e4, isOutput=False)
    # Host-prebroadcast sq2 ([P, N2], all rows equal), fp16: a device-side
    # partition-broadcast DMA lands on a single engine and takes ~20 us; a
    # plain contiguous read spreads over all 16 DMA engines.
    sq2 = nc.declare_dram_parameter("sq2", [P, N2], mybir.dt.float16, isOutput=False)
    b1 = nc.declare_dram_parameter("b1", [P, NT], mybir.dt.float32, isOutput=False)
    out = nc.declare_dram_parameter("out", [ROWS, N2], mybir.dt.float16, isOutput=True)

    with tile.TileContext(nc) as tc:
        with (
            tc.tile_pool(name="singles", bufs=1) as singles,
            tc.tile_pool(name="x2pool", bufs=16) as x2pool,
            tc.tile_pool(name="psum", bufs=4, space="PSUM") as psumpool,
            # Deep epilogue rings: shallow ones (bufs=4) made ACT(g) wait on
            # the out-DMA transfer of g-4, collapsing the PE pipeline behind
            # DMA latency every ~10 us and re-triggering the pstate ramp.
            tc.tile_pool(name="tpool", bufs=12) as tpool,
            tc.tile_pool(name="opool", bufs=12) as opool,
            tc.tile_pool(name="twide", bufs=6) as twide,
            tc.tile_pool(name="owide", bufs=6) as owide,
        ):
            b1sb = singles.tile([P, NT], mybir.dt.float32)
            x1sb = [
                singles.tile([P, NT, 2, P], mybir.dt.float8e4, tag=f"x1k{kk}", name=f"x1k{kk}")
                for kk in range(KT8)
            ]
            # sq2 SBUF: m2=0 as two [P, MB] half tiles (the h0 half rides
            # 2nd on the scalar ring so the FIRST epilogue isn't sq2-gated);
            # m2>=1 as one [P, MB2] tile each (one 256 KB DMA, 2 KB lines).
            sq2m0h = [
                singles.tile([P, MB], mybir.dt.float16, tag=f"sq2m0h{h}", name=f"sq2m0h{h}")
                for h in range(2)
            ]
            sq2sb = [None] + [
                singles.tile([P, MB2], mybir.dt.float16, tag=f"sq2m{mq}", name=f"sq2m{mq}")
                for mq in range(1, MT2)
            ]

            def new_x2_tile(m2, kk, h):
                return x2pool.tile(
                    [P, 2, MB], mybir.dt.float8e4, tag="x2", name=f"x2_{m2}_{kk}_{h}"
                )

            def load_x2(eng, t, m2, kk, h):
                eng.dma_start(out=t[:], in_=x2t[m2, kk, h])

            HN = NT // 2

            def load_x1_half(eng, kk, hh):
                eng.dma_start(
                    out=x1sb[kk][:, hh * HN:(hh + 1) * HN, :, :],
                    in_=x1t[kk, :, hh * HN:(hh + 1) * HN, :, :],
                )

            def load_sq2(eng, mq):
                eng.dma_start(
                    out=sq2sb[mq][:], in_=sq2[:, mq * MB2:(mq + 1) * MB2]
                )

            # (No PE warmup: the pstate ramp is hidden behind the head-load
            # bandwidth wall, and pre-warming inflates the PE semaphore
            # counter -- the scheduler's coarsened sem-ge targets for the
            # first epilogue adds then sit at the PSUM-backpressure limit.)

            # Head order = dispatch priority = first-use order of the m2=0
            # cohorts (measured: first packet ~1.5 us after dispatch, then
            # ~80-280 GB/s per queue depending on how many queues pull).
            # Every load lands just before its first consuming pass; no
            # non-critical byte rides ahead of a critical one.
            # Head schedule. Measured reality: the 3 DGE queues share 16 DMA
            # engines (~0.25-0.30 MB/us aggregate in the head window), and
            # the tile scheduler COARSENS a matmul's DMA-completion waits to
            # a later ring count (a pass that needs the ring's 2nd tile can
            # end up waiting its ~5th), so fine-grained just-in-time
            # ordering is defeated: the PE stutters, and every multi-us
            # stall re-triggers the ~3 us half-speed pstate ramp. Strategy
            # instead: consolidate ALL head waiting into one pre-start GATE
            # (below) so the PE starts once, ramps once, and runs the 64
            # m2=0 passes stall-free.
            x2cur = {(kk, h): new_x2_tile(0, kk, h) for kk in range(KT8) for h in range(2)}
            # sync: the four h0 rhs tiles + bias, nothing else ahead of the
            # out-DMA stream (keeps the gate's coarsened target ~<=5)
            for kk in range(KT8):
                load_x2(nc.sync, x2cur[(kk, 0)], 0, kk, 0)
            nc.sync.dma_start(out=b1sb[:], in_=b1[:, :])
            # scalar: the x1 A-halves (cohort A), then the first two h1 rhs
            # tiles (passes 33-40); the last two h1 tiles ride sync behind
            # b1, where the ring has drained by then
            for kk in range(KT8):
                load_x1_half(nc.scalar, kk, 0)
            load_x2(nc.scalar, x2cur[(0, 1)], 0, 0, 1)
            load_x2(nc.scalar, x2cur[(1, 1)], 0, 1, 1)
            load_x2(nc.sync, x2cur[(2, 1)], 0, 2, 1)
            load_x2(nc.sync, x2cur[(3, 1)], 0, 3, 1)
            # gpsimd: m2=0 sq2 halves (first epilogue ~pass 13), x1
            # B-halves (passes 17-29), then the m2=1 prefetch follows.
            nc.gpsimd.dma_start(out=sq2m0h[0][:], in_=sq2[:, 0:MB])
            nc.gpsimd.dma_start(out=sq2m0h[1][:], in_=sq2[:, MB:MB2])
            for kk in range(KT8):
                load_x1_half(nc.gpsimd, kk, 1)

            def sq2_ap(m2, h, lo, hi):
                if m2 == 0:
                    return sq2m0h[h][:, lo:hi]
                return sq2sb[m2][:, h * MB + lo:h * MB + hi]

            def epilogue(ps, m2, n, h, bounds=((0, MB),), dma_engs=None):
                pt, cb = ps
                t = tpool.tile([P, MB], mybir.dt.float32)
                o = opool.tile([P, MB], mybir.dt.float16)
                base = m2 * MB2 + h * MB
                for ci, (lo, hi) in enumerate(bounds):
                    nc.vector.tensor_add(t[:, lo:hi], pt[:, cb + lo:cb + hi], sq2_ap(m2, h, lo, hi))
                    nc.scalar.activation(
                        out=o[:, lo:hi],
                        in_=t[:, lo:hi],
                        func=mybir.ActivationFunctionType.Ln,
                        bias=b1sb[:, n:n + 1],
                        scale=1.0,
                    )
                    eng = dma_engs[ci] if dma_engs else nc.sync
                    eng.dma_start(
                        out=out[n * P:(n + 1) * P, base + lo:base + hi],
                        in_=o[:, lo:hi],
                    )

            for m2 in range(MT2):
                x2m = x2cur
                x2nxt = None
                if m2 + 1 < MT2:
                    # m2+1 prefetch on gpsimd SWDGE (own DMASW sem lanes --
                    # a HWDGE ring would couple it to out-DMA lane turnover)
                    x2nxt = {}
                    for kk in range(KT8):
                        for h in range(2):
                            tn = new_x2_tile(m2 + 1, kk, h)
                            load_x2(nc.gpsimd, tn, m2 + 1, kk, h)
                            x2nxt[(kk, h)] = tn
                            if kk == 1 and h == 1:
                                load_sq2(nc.gpsimd, m2 + 1)
                if 0 < m2 < MT2 - 1:
                    # steady state: v2-style 1024-wide groups (half the
                    # epilogue instruction count -- 512-wide everywhere
                    # pushed the ACT engine to 110 us, co-bottlenecking
                    # with the PE's 113); kk outer / h inner so both
                    # 512-col passes stream against stationary weights.
                    for n in range(NT):
                        psw = psumpool.tile(
                            [P, MB2], mybir.dt.float32,
                            tag="ps", name=f"psw_{m2}_{n}",
                        )
                        for kk in range(KT8):
                            for h in range(2):
                                nc.tensor.matmul(
                                    psw[:, h * MB:(h + 1) * MB],
                                    lhsT=x1sb[kk][:, n, :, :],
                                    rhs=x2m[(kk, h)][:],
                                    start=(kk == 0),
                                    stop=(kk == KT8 - 1),
                                    skip_group_check=True,
                                    perf_mode=mybir.MatmulPerfMode.DoubleRowSwInterleave,
                                )
                        tw = twide.tile([P, MB2], mybir.dt.float32, tag="tw", name=f"tw_{m2}_{n}")
                        ow = owide.tile([P, MB2], mybir.dt.float16, tag="ow", name=f"ow_{m2}_{n}")
                        nc.vector.tensor_add(tw[:], psw[:], sq2sb[m2][:])
                        nc.scalar.activation(
                            out=ow[:],
                            in_=tw[:],
                            func=mybir.ActivationFunctionType.Ln,
                            bias=b1sb[:, n:n + 1],
                            scale=1.0,
                        )
                        nc.sync.dma_start(
                            out=out[n * P:(n + 1) * P, m2 * MB2:(m2 + 1) * MB2],
                            in_=ow[:],
                        )
                    if x2nxt is not None:
                        x2cur = x2nxt
                    continue
                last_slice_h1 = (m2 == MT2 - 1)
                for h in range(2):
                    if last_slice_h1 and h == 1:
                        # drain-friendly cohorts: the n4-6 epilogues overlap
                        # the final 4 passes; the last group drains alone
                        cohorts = [(0, 1, 2, 3), (4, 5, 6), (7,)]
                    else:
                        cohorts = [(0, 1, 2, 3), (4, 5, 6, 7)]
                    for ns in cohorts:
                        # PSUM slots are 2-bank granular: allocate [P, MB2]
                        # tiles and give each group one independent 512-wide
                        # half (separate accumulation region, region-granular
                        # deps) -- 4 tiles in the pool = 8 groups in flight.
                        ptiles = [
                            psumpool.tile(
                                [P, MB2], mybir.dt.float32,
                                tag="ps", name=f"ps_{m2}_{h}_{ns[0]}_{i}",
                            )
                            for i in range((len(ns) + 1) // 2)
                        ]
                        pss = {
                            n: (ptiles[i // 2], (i % 2) * MB)
                            for i, n in enumerate(ns)
                        }
                        if m2 == 0 and h == 0 and ns[0] == 0:
                            # GATE: a 16-column matmul reading the LAST
                            # h0/x1A head tiles into a dead psum region
                            # (immediately re-zeroed by the kk0 start pass).
                            # It delays the PE start until the whole h0
                            # working set is resident, so the 64-pass m2=0
                            # stream runs with ONE ramp and no mid-stream
                            # stalls (starting at first-tile-arrival gained
                            # 3 us of start but lost 5+ to stutter+re-ramp).
                            # Gate on the 2nd wave (ring delivery is ~2 us
                            # per 128 KB tile, so w2 ~13.5): passes 1-8's
                            # operands are then resident, and the ramp-slowed
                            # early passes (~427 ns) put pass 9/13's natural
                            # times (~17.2/18.1) behind waves 3/4 (~15.5/
                            # 17.5) even if their waits are coarsened.
                            nc.tensor.matmul(
                                ptiles[0][:, 0:16],
                                lhsT=x1sb[1][:, 1, :, :],
                                rhs=x2cur[(1, 0)][:, :, 0:16],
                                start=True,
                                stop=True,
                                skip_group_check=True,
                                perf_mode=mybir.MatmulPerfMode.DoubleRowSwInterleave,
                            )
                        # kk outer / n inner: each new 128 KB x2 half (and
                        # 128 KB x1 half) unlocks 4 more passes
                        for kk in range(KT8):
                            for n in ns:
                                pt, cb = pss[n]
                                nc.tensor.matmul(
                                    pt[:, cb:cb + MB],
                                    lhsT=x1sb[kk][:, n, :, :],
                                    rhs=x2m[(kk, h)][:],
                                    start=(kk == 0),
                                    stop=(kk == KT8 - 1),
                                    skip_group_check=True,
                                    perf_mode=mybir.MatmulPerfMode.DoubleRowSwInterleave,
                                )
                        for n in ns:
                            final = last_slice_h1 and h == 1 and n == NT - 1
                            if final:
                                # chunked drain; both chunk DMAs on HWDGE
                                # queues -- a gpsimd (SWDGE) out-DMA here
                                # put its ~7 us queue drain on the tail
                                QB = MB // 2
                                epilogue(
                                    pss[n], m2, n, h,
                                    bounds=((0, QB), (QB, MB)),
                                    dma_engs=[nc.sync, nc.scalar],
                                )
                            else:
                                epilogue(pss[n], m2, n, h)
                if x2nxt is not None:
                    x2cur = x2nxt
    if split_waits:
        _split_sync_waits(nc)
    return nc


def kernel(x1, x2, _trace=False):
    global _nc_cache, last_results
    x1f = np.asarray(x1, dtype=np.float32)
    x2f = np.asarray(x2, dtype=np.float32)
    assert x1f.shape == (N1, D) and x2f.shape == (N2, D)

    a8 = (-2.0 * x1f).astype(F8)                # [N1, D] fp8(-2 x1)
    x2_8 = x2f.astype(F8)                       # [N2, D]
    x1ts = np.ascontiguousarray(a8.T).reshape(KT8, P, 2, N1)
    # [KT8, P, 2, N2] -> h-major [MT2, KT8, 2, P, 2, MB] (one contiguous
    # 128 KB block per (m2, kk, h) device tile)
    x2t = np.ascontiguousarray(
        x2_8.T.reshape(KT8, P, 2, MT2, 2, MB).transpose(3, 0, 4, 1, 2, 5)
    )
    # SwInterleave weight layout: per 128-column block, pairs (j=0, j=1)
    # interleaved per column with columns reversed:
    # flat[q] with q = 2*(127-c) + j  <->  logical[j, c]
    g = x1ts.reshape(KT8, P, 2, N1 // P, P)           # [kk, p, j, nblk, c]
    g = g[:, :, :, :, ::-1].transpose(0, 1, 3, 4, 2)  # [kk, p, nblk, c~, j]
    x1ts = np.ascontiguousarray(g).reshape(KT8, P, N1 // P, 2, P)

    sq1 = (x1f.astype(np.float64) ** 2).sum(axis=-1)
    sq2 = (x2f.astype(np.float64) ** 2).sum(axis=-1)
    bias1 = (1.0 + sq1).astype(np.float32)        # [N1]
    # host-side partition broadcast (see sq2 dram param comment), fp16
    sq2_bc = np.ascontiguousarray(
        np.broadcast_to(sq2.astype(np.float16).reshape(1, N2), (P, N2))
    )

    in_maps = []
    for c in range(N_CORES):
        r0, r1 = c * ROWS, (c + 1) * ROWS
        in_maps.append({
            "x1t": np.ascontiguousarray(x1ts[:, :, c * NT:(c + 1) * NT]),
            "x2t": x2t,
            "sq2": sq2_bc,
            # b1[p, n] = 1 + sq1[r0 + n*128 + p]
            "b1": np.ascontiguousarray(bias1[r0:r1].reshape(NT, P).T),
        })

    if _nc_cache is None:
        _nc_cache = _build_nc()
    res = None
    for attempt in range(3):
        try:
            res = run_bass_kernel_spmd(
                _nc_cache, in_maps, core_ids=list(range(N_CORES)), trace=_trace
            )
            break
        except Exception:
            if attempt == 2:
                raise
            time.sleep(5.0)
    last_results = res
    # Device computes +log1p(d2) in fp16; the sign flip and f32 upcast are
    # part of the host-side unshard.
    full = np.concatenate([res.results[c]["out"] for c in range(N_CORES)], axis=0)
    return -full.astype(np.float32)


# revision 21
# speedup vs baseline: 1.0324x; 1.0041x over previous
"""Pairwise squared-euclidean-distance kernel (-log1p(max(d2,0))) for 8 trn2 cores.

Strategy (sharding_hint): shard x1 rows across the 8 NeuronCores (1024 rows
each); replicate x2. Each core computes a [1024, 8192] slab of the output:

    out[n, m] = -log1p(sq1[n] + sq2[m] - 2 * x1[n] . x2[m])

Device work per core: a [1024 x 1024] @ [1024 x 8192] matmul into PSUM
(psum = -2 * cross, the -2 baked into the lhsT operand on the host; fp8 e4m3
operands, DoubleRowSwInterleave so each 512-col pass covers 256 contraction
rows -- the HW-max rate of 1 moving column/cycle at 2.4 GHz, ~216 ns/pass),
then per 512-wide psum group an epilogue:
    DVE: t = psum + sq2_slice            (sq2 varies along the free dim)
    ACT: o = Ln(t + (1 + sq1[n]))        (per-partition bias), fp16 out
The negate and the fp16->fp32 upcast happen on the HOST during the unshard.

v3 over v2 (v2 = fp8/fp16-out, 1024-wide psum tiles, kk-outer/h-inner):
the PE stream was gap-free at the fp8 roofline, but ntff DMA-packet analysis
showed the ends are HBM-arrival-bound: the three DGE queues (sync/scalar HW,
gpsimd SW) share 16 DMA engines (~340 GB/s), first packet lands ~9.2 us
(after the fixed ~7.5 us NEFF preamble) and v2's pass order needed 1.5 MB
in-flow by pass 8 -> last matmul at ~126 us; the 1024-wide final epilogue +
single-ring drain added ~7.7 us of tail. v3:
  - 512-wide psum groups everywhere (16 banks' worth in 8 [P,512] bufs), in
    4-group COHORTS with kk outer / n inner: cohort 1 of m2=0 needs only
    x1[kk0,n0-3] + x2[m2=0,kk0,h0] (256 KB) to start and adds 128 KB per
    4 passes -- the in-flow requirement grows slower than DMA arrival, so
    the PE never waits on HBM after pass 1 (v2 stalled the whole first
    n-tile on the 1.5 MB wall).
  - x2 DRAM h-major ([MT2, KT8, 2, P, 2, MB]): each (kk, h) 512-col half is
    one contiguous 128 KB read with a 1 KB line per partition.
  - sq2 in fp16 (output error ~1e-4): halves sq2 bytes; m2=0's sq2 is two
    [P, 512] tiles loaded 2nd on the scalar ring, so the first epilogue
    fires right at its psum stop instead of 6 us late (v2 loaded sq2 last,
    eating the entire psum-buffer slack).
  - tail: the final slice's h1 groups run as cohorts {n0-3}, {n4-6}, {n7}:
    the n4-6 epilogues drain behind the last 4 matmul passes, and the final
    group drains in 2x256 chunks with out-DMAs on two different queues.
sq1/sq2 are computed on the host in float64 from the exact inputs (0.01% of
total FLOPs); all N1*N2*D matmul work runs on the NeuronCores.
"""

import time

import numpy as np
import ml_dtypes

import bass_rust
import concourse.bass as bass
import concourse.mybir as mybir
import concourse.tile as tile
from concourse.bass_utils import run_bass_kernel_spmd

# ---------------------------------------------------------------------------
# The pinned walrus rejects instructions carrying more than a small number
# of sem-wait commands ("Too many sync wait commands", CoreV3GenImpl
# setupSyncWait): a drain with 3 waits and a TensorTensor with 3 waits both
# fail; only 1 wait compiles. Post-pass: move excess waits onto NoOp
# instructions inserted immediately before the offender on the same engine
# queue -- waits accumulate across adjacent instructions, so semantics are
# unchanged.
_MAX_WAITS = 1

_split_counter = [0]


def _split_sync_waits(nc, limit=_MAX_WAITS):
    n_split = 0
    for f in nc.m.functions:
        for bb in f.blocks:
            insts = bb.instructions
            out = []
            changed = False
            for inst in insts:
                si = inst.sync_info
                waits = list(si.on_wait) if si and si.on_wait else []
                lim = 1 if inst.engine == mybir.EngineType.SP else limit
                if len(waits) > lim:
                    changed = True
                    n_split += 1
                    excess, keep = waits[:-lim], waits[-lim:]
                    si.on_wait = keep
                    for i in range(0, len(excess), lim):
                        _split_counter[0] += 1
                        nop = mybir.InstNoOp(
                            name=f"I-waitsplit-{_split_counter[0]}",
                            engine=inst.engine,
                            ins=[],
                            outs=[],
                            bass_nofuse=True,
                            sync_info=bass_rust.SyncInfo(
                                on_wait=excess[i:i + lim], on_update=[]
                            ),
                        )
                        out.append(nop)
                out.append(inst)
            if changed:
                bb.instructions = out
    return n_split


N1, N2, D = 8192, 8192, 1024
N_CORES = 8
ROWS = N1 // N_CORES  # 1024 x1 rows per core
P = 128               # SBUF/PSUM partitions
NT = ROWS // P        # 8 n-tiles (output partition tiles) per core
MB = 512              # one fp32 PSUM bank = one psum group width
KT8 = D // 256        # 4 DoubleRow super k-tiles (256 contraction rows each)
MB2 = 2 * MB          # 1024-col m2 slice (2 h-halves)
MT2 = N2 // MB2       # 8 m2 slices
F8 = ml_dtypes.float8_e4m3
F16 = np.float16

_nc_cache = None
last_results = None


def _build_nc(split_waits=True):
    """fp8 e4m3 DoubleRowSwInterleave: 2 contraction rows per PE cell,
    weights pre-interleaved on the host so LDWEIGHTS streams contiguously.

    Operand layout: K = kk*256 + 2*p + j maps contraction row K to
    (partition p, pair-slot j) of super-tile kk on BOTH operands, so
    out[n, m] = sum_{p,j} lhsT[p, j, n] * rhs[p, j, m] is the plain dot
    product. Host arrays are reshaped [D, X] -> [KT8, 128, 2, X] (x1
    additionally SW-interleaved, see kernel()).
    """
    nc = bass.Bass()
    x1t = nc.declare_dram_parameter("x1t", [KT8, P, NT, 2, P], mybir.dt.float8e4, isOutput=False)
    # h-major: each (m2, kk, h) 512-col half is one contiguous 128 KB block
    x2t = nc.declare_dram_parameter("x2t", [MT2, KT8, 2, P, 2, MB], mybir.dt.float8e4, isOutput=False)
    # Host-prebroadcast sq2 ([P, N2], all rows equal), fp16: a device-side
    # partition-broadcast DMA lands on a single engine and takes ~20 us; a
    # plain contiguous read spreads over all 16 DMA engines.
    sq2 = nc.declare_dram_parameter("sq2", [P, N2], mybir.dt.float16, isOutput=False)
    b1 = nc.declare_dram_parameter("b1", [P, NT], mybir.dt.float32, isOutput=False)
    out = nc.declare_dram_parameter("out", [ROWS, N2], mybir.dt.float16, isOutput=True)

    with tile.TileContext(nc) as tc:
        with (
            tc.tile_pool(name="singles", bufs=1) as singles,
            tc.tile_pool(name="x2pool", bufs=16) as x2pool,
            tc.tile_pool(name="psum", bufs=4, space="PSUM") as psumpool,
            # Deep epilogue rings: shallow ones (bufs=4) made ACT(g) wait on
            # the out-DMA transfer of g-4, collapsing the PE pipeline behind
            # DMA latency every ~10 us and re-triggering the pstate ramp.
            tc.tile_pool(name="tpool", bufs=12) as tpool,
            tc.tile_pool(name="opool", bufs=12) as opool,
            tc.tile_pool(name="twide", bufs=6) as twide,
            tc.tile_pool(name="owide", bufs=6) as owide,
        ):
            b1sb = singles.tile([P, NT], mybir.dt.float32)
            x1sb = [
                singles.tile([P, NT, 2, P], mybir.dt.float8e4, tag=f"x1k{kk}", name=f"x1k{kk}")
                for kk in range(KT8)
            ]
            # sq2 SBUF: m2=0 as two [P, MB] half tiles (the h0 half rides
            # 2nd on the scalar ring so the FIRST epilogue isn't sq2-gated);
            # m2>=1 as one [P, MB2] tile each (one 256 KB DMA, 2 KB lines).
            sq2m0h = [
                singles.tile([P, MB], mybir.dt.float16, tag=f"sq2m0h{h}", name=f"sq2m0h{h}")
                for h in range(2)
            ]
            sq2sb = [None] + [
                singles.tile([P, MB2], mybir.dt.float16, tag=f"sq2m{mq}", name=f"sq2m{mq}")
                for mq in range(1, MT2)
            ]

            def new_x2_tile(m2, kk, h):
                return x2pool.tile(
                    [P, 2, MB], mybir.dt.float8e4, tag="x2", name=f"x2_{m2}_{kk}_{h}"
                )

            def load_x2(eng, t, m2, kk, h):
                eng.dma_start(out=t[:], in_=x2t[m2, kk, h])

            HN = NT // 2

            def load_x1_half(eng, kk, hh):
                eng.dma_start(
                    out=x1sb[kk][:, hh * HN:(hh + 1) * HN, :, :],
                    in_=x1t[kk, :, hh * HN:(hh + 1) * HN, :, :],
                )

            def load_sq2(eng, mq):
                eng.dma_start(
                    out=sq2sb[mq][:], in_=sq2[:, mq * MB2:(mq + 1) * MB2]
                )

            # (No PE warmup: the pstate ramp is hidden behind the head-load
            # bandwidth wall, and pre-warming inflates the PE semaphore
            # counter -- the scheduler's coarsened sem-ge targets for the
            # first epilogue adds then sit at the PSUM-backpressure limit.)

            # Head order = dispatch priority = first-use order of the m2=0
            # cohorts (measured: first packet ~1.5 us after dispatch, then
            # ~80-280 GB/s per queue depending on how many queues pull).
            # Every load lands just before its first consuming pass; no
            # non-critical byte rides ahead of a critical one.
            # Head schedule. Measured reality: the 3 DGE queues share 16 DMA
            # engines (~0.25-0.30 MB/us aggregate in the head window), and
            # the tile scheduler COARSENS a matmul's DMA-completion waits to
            # a later ring count (a pass that needs the ring's 2nd tile can
            # end up waiting its ~5th), so fine-grained just-in-time
            # ordering is defeated: the PE stutters, and every multi-us
            # stall re-triggers the ~3 us half-speed pstate ramp. Strategy
            # instead: consolidate ALL head waiting into one pre-start GATE
            # (below) so the PE starts once, ramps once, and runs the 64
            # m2=0 passes stall-free.
            x2cur = {(kk, h): new_x2_tile(0, kk, h) for kk in range(KT8) for h in range(2)}
            # sync: the four h0 rhs tiles + bias, nothing else ahead of the
            # out-DMA stream (keeps the gate's coarsened target ~<=5)
            for kk in range(KT8):
                load_x2(nc.sync, x2cur[(kk, 0)], 0, kk, 0)
            nc.sync.dma_start(out=b1sb[:], in_=b1[:, :])
            # scalar: the x1 A-halves (cohort A), then the first two h1 rhs
            # tiles (passes 33-40); the last two h1 tiles ride sync behind
            # b1, where the ring has drained by then
            for kk in range(KT8):
                load_x1_half(nc.scalar, kk, 0)
            load_x2(nc.scalar, x2cur[(0, 1)], 0, 0, 1)
            load_x2(nc.scalar, x2cur[(1, 1)], 0, 1, 1)
            load_x2(nc.sync, x2cur[(2, 1)], 0, 2, 1)
            load_x2(nc.sync, x2cur[(3, 1)], 0, 3, 1)
            # gpsimd: m2=0 sq2 halves (first epilogue ~pass 13), x1
            # B-halves (passes 17-29), then the m2=1 prefetch follows.
            nc.gpsimd.dma_start(out=sq2m0h[0][:], in_=sq2[:, 0:MB])
            nc.gpsimd.dma_start(out=sq2m0h[1][:], in_=sq2[:, MB:MB2])
            for kk in range(KT8):
                load_x1_half(nc.gpsimd, kk, 1)

            def sq2_ap(m2, h, lo, hi):
                if m2 == 0:
                    return sq2m0h[h][:, lo:hi]
                return sq2sb[m2][:, h * MB + lo:h * MB + hi]

            def epilogue(ps, m2, n, h, bounds=((0, MB),), dma_engs=None):
                pt, cb = ps
                t = tpool.tile([P, MB], mybir.dt.float32)
                o = opool.tile([P, MB], mybir.dt.float16)
                base = m2 * MB2 + h * MB
                for ci, (lo, hi) in enumerate(bounds):
                    nc.vector.tensor_add(t[:, lo:hi], pt[:, cb + lo:cb + hi], sq2_ap(m2, h, lo, hi))
                    nc.scalar.activation(
                        out=o[:, lo:hi],
                        in_=t[:, lo:hi],
                        func=mybir.ActivationFunctionType.Ln,
                        bias=b1sb[:, n:n + 1],
                        scale=1.0,
                    )
                    eng = dma_engs[ci] if dma_engs else nc.sync
                    eng.dma_start(
                        out=out[n * P:(n + 1) * P, base + lo:base + hi],
                        in_=o[:, lo:hi],
                    )

            for m2 in range(MT2):
                x2m = x2cur
                x2nxt = None
                if m2 + 1 < MT2:
                    # m2+1 prefetch on gpsimd SWDGE (own DMASW sem lanes --
                    # a HWDGE ring would couple it to out-DMA lane turnover)
                    x2nxt = {}
                    for kk in range(KT8):
                        for h in range(2):
                            tn = new_x2_tile(m2 + 1, kk, h)
                            load_x2(nc.gpsimd, tn, m2 + 1, kk, h)
                            x2nxt[(kk, h)] = tn
                            if kk == 1 and h == 1:
                                load_sq2(nc.gpsimd, m2 + 1)
                if 0 < m2 < MT2 - 1:
                    # steady state: v2-style 1024-wide groups (half the
                    # epilogue instruction count -- 512-wide everywhere
                    # pushed the ACT engine to 110 us, co-bottlenecking
                    # with the PE's 113); kk outer / h inner so both
                    # 512-col passes stream against stationary weights.
                    for n in range(NT):
                        psw = psumpool.tile(
                            [P, MB2], mybir.dt.float32,
                            tag="ps", name=f"psw_{m2}_{n}",
                        )
                        for kk in range(KT8):
                            for h in range(2):
                                nc.tensor.matmul(
                                    psw[:, h * MB:(h + 1) * MB],
                                    lhsT=x1sb[kk][:, n, :, :],
                                    rhs=x2m[(kk, h)][:],
                                    start=(kk == 0),
                                    stop=(kk == KT8 - 1),
                                    skip_group_check=True,
                                    perf_mode=mybir.MatmulPerfMode.DoubleRowSwInterleave,
                                )
                        tw = twide.tile([P, MB2], mybir.dt.float32, tag="tw", name=f"tw_{m2}_{n}")
                        ow = owide.tile([P, MB2], mybir.dt.float16, tag="ow", name=f"ow_{m2}_{n}")
                        nc.vector.tensor_add(tw[:], psw[:], sq2sb[m2][:])
                        nc.scalar.activation(
                            out=ow[:],
                            in_=tw[:],
                            func=mybir.ActivationFunctionType.Ln,
                            bias=b1sb[:, n:n + 1],
                            scale=1.0,
                        )
                        nc.sync.dma_start(
                            out=out[n * P:(n + 1) * P, m2 * MB2:(m2 + 1) * MB2],
                            in_=ow[:],
                        )
                    if x2nxt is not None:
                        x2cur = x2nxt
                    continue
                last_slice_h1 = (m2 == MT2 - 1)
                for h in range(2):
                    if last_slice_h1 and h == 1:
                        # drain-friendly cohorts: each small cohort's
                        # epilogues drain behind the next cohort's passes,
                        # and the last group drains alone (a 4-group final
                        # cohort left ~1.3 us of ACT overhang on the tail)
                        cohorts = [(0, 1, 2, 3), (4, 5), (6,), (7,)]
                    else:
                        cohorts = [(0, 1, 2, 3), (4, 5, 6, 7)]
                    for ns in cohorts:
                        # PSUM slots are 2-bank granular: allocate [P, MB2]
                        # tiles and give each group one independent 512-wide
                        # half (separate accumulation region, region-granular
                        # deps) -- 4 tiles in the pool = 8 groups in flight.
                        ptiles = [
                            psumpool.tile(
                                [P, MB2], mybir.dt.float32,
                                tag="ps", name=f"ps_{m2}_{h}_{ns[0]}_{i}",
                            )
                            for i in range((len(ns) + 1) // 2)
                        ]
                        pss = {
                            n: (ptiles[i // 2], (i % 2) * MB)
                            for i, n in enumerate(ns)
                        }
                        if m2 == 0 and h == 0 and ns[0] == 0:
                            # GATE: a 16-column matmul reading the LAST
                            # h0/x1A head tiles into a dead psum region
                            # (immediately re-zeroed by the kk0 start pass).
                            # It delays the PE start until the whole h0
                            # working set is resident, so the 64-pass m2=0
                            # stream runs with ONE ramp and no mid-stream
                            # stalls (starting at first-tile-arrival gained
                            # 3 us of start but lost 5+ to stutter+re-ramp).
                            # Gate on the 2nd wave (ring delivery is ~2 us
                            # per 128 KB tile, so w2 ~13.5): passes 1-8's
                            # operands are then resident, and the ramp-slowed
                            # early passes (~427 ns) put pass 9/13's natural
                            # times (~17.2/18.1) behind waves 3/4 (~15.5/
                            # 17.5) even if their waits are coarsened.
                            nc.tensor.matmul(
                                ptiles[0][:, 0:16],
                                lhsT=x1sb[1][:, 1, :, :],
                                rhs=x2cur[(1, 0)][:, :, 0:16],
                                start=True,
                                stop=True,
                                skip_group_check=True,
                                perf_mode=mybir.MatmulPerfMode.DoubleRowSwInterleave,
                            )
                        # kk outer / n inner: each new 128 KB x2 half (and
                        # 128 KB x1 half) unlocks 4 more passes
                        for kk in range(KT8):
                            for n in ns:
                                pt, cb = pss[n]
                                nc.tensor.matmul(
                                    pt[:, cb:cb + MB],
                                    lhsT=x1sb[kk][:, n, :, :],
                                    rhs=x2m[(kk, h)][:],
                                    start=(kk == 0),
                                    stop=(kk == KT8 - 1),
                                    skip_group_check=True,
                                    perf_mode=mybir.MatmulPerfMode.DoubleRowSwInterleave,
                                )
                        for n in ns:
                            final = last_slice_h1 and h == 1 and n == NT - 1
                            if final:
                                # chunked drain; both chunk DMAs on HWDGE
                                # queues -- a gpsimd (SWDGE) out-DMA here
                                # put its ~7 us queue drain on the tail
                                QB = MB // 2
                                epilogue(
                                    pss[n], m2, n, h,
                                    bounds=((0, QB), (QB, MB)),
                                    dma_engs=[nc.sync, nc.scalar],
                                )
                            else:
                                epilogue(pss[n], m2, n, h)
                if x2nxt is not None:
                    x2cur = x2nxt
    if split_waits:
        _split_sync_waits(nc)
    return nc


def kernel(x1, x2, _trace=False):
    global _nc_cache, last_results
    x1f = np.asarray(x1, dtype=np.float32)
    x2f = np.asarray(x2, dtype=np.float32)
    assert x1f.shape == (N1, D) and x2f.shape == (N2, D)

    a8 = (-2.0 * x1f).astype(F8)                # [N1, D] fp8(-2 x1)
    x2_8 = x2f.astype(F8)                       # [N2, D]
    x1ts = np.ascontiguousarray(a8.T).reshape(KT8, P, 2, N1)
    # [KT8, P, 2, N2] -> h-major [MT2, KT8, 2, P, 2, MB] (one contiguous
    # 128 KB block per (m2, kk, h) device tile)
    x2t = np.ascontiguousarray(
        x2_8.T.reshape(KT8, P, 2, MT2, 2, MB).transpose(3, 0, 4, 1, 2, 5)
    )
    # SwInterleave weight layout: per 128-column block, pairs (j=0, j=1)
    # interleaved per column with columns reversed:
    # flat[q] with q = 2*(127-c) + j  <->  logical[j, c]
    g = x1ts.reshape(KT8, P, 2, N1 // P, P)           # [kk, p, j, nblk, c]
    g = g[:, :, :, :, ::-1].transpose(0, 1, 3, 4, 2)  # [kk, p, nblk, c~, j]
    x1ts = np.ascontiguousarray(g).reshape(KT8, P, N1 // P, 2, P)

    sq1 = (x1f.astype(np.float64) ** 2).sum(axis=-1)
    sq2 = (x2f.astype(np.float64) ** 2).sum(axis=-1)
    bias1 = (1.0 + sq1).astype(np.float32)        # [N1]
    # host-side partition broadcast (see sq2 dram param comment), fp16
    sq2_bc = np.ascontiguousarray(
        np.broadcast_to(sq2.astype(np.float16).reshape(1, N2), (P, N2))
    )

    in_maps = []
    for c in range(N_CORES):
        r0, r1 = c * ROWS, (c + 1) * ROWS
        in_maps.append({
            "x1t": np.ascontiguousarray(x1ts[:, :, c * NT:(c + 1) * NT]),
            "x2t": x2t,
            "sq2": sq2_bc,
            # b1[p, n] = 1 + sq1[r0 + n*128 + p]
            "b1": np.ascontiguousarray(bias1[r0:r1].reshape(NT, P).T),
        })

    if _nc_cache is None:
        _nc_cache = _build_nc()
    res = None
    for attempt in range(3):
        try:
            res = run_bass_kernel_spmd(
                _nc_cache, in_maps, core_ids=list(range(N_CORES)), trace=_trace
            )
            break
        except Exception:
            if attempt == 2:
                raise
            time.sleep(5.0)
    last_results = res
    # Device computes +log1p(d2) in fp16; the sign flip and f32 upcast are
    # part of the host-side unshard.
    full = np.concatenate([res.results[c]["out"] for c in range(N_CORES)], axis=0)
    return -full.astype(np.float32)
